# revision 59
# baseline (speedup 1.0000x reference)
"""Trainium2 Bass kernel for nn_GUARDIAN_69312182223528 (gnn_message_passing).

Full-input contract: kernel(**inputs) -> np.ndarray [8000, 512].

Strategy (8 NeuronCores, SPMD single NEFF):
- Nodes are dealt to 8 cores balanced by (in-degree, out-degree); each core is
  padded with a few fake nodes/edges so all cores share ONE degree profile and
  hence one static LSTM schedule.
- Per core, per aggregator (in = bucket by dst, out = bucket by src), the host
  builds a step-major edge permutation: step t holds the position-t edge of
  every slot (node) with degree > t; slots are sorted by degree descending so
  the active set at each step is a shrinking prefix.
- Host marshals per-core inputs: attrT [8, Ec512] (edge attrs, transposed,
  bf16) and posT [128, 2*Ec512] (time_scale * pos_emb[positions], transposed
  to feature-on-partition, bf16) -- the same class of host marshaling as the
  edge permutation itself.
- Device ef build: psum = W_proj @ attrT chunk (bf16 matmul), then DVE adds
  posT in-place (posT tile becomes efT).
- Recurrence: all matmul operands bf16 (1 cyc/col on PE at any width), cell
  state c stays f32. COLTILE=256 with four rotating 2-bank PSUM tiles gives
  per-chain double buffering: next coltile's Wih matmuls issue while the
  previous tile drains through sigmoid/tanh. Per step, fwd and bwd chains
  interleave at coltile granularity; per gate-tile PSUM order is i,f,o,g at
  256-col offsets.
- Degree-1 nodes bypass the LSTM (out = ef of their single edge); degree-0
  stay zero.
- out-aggregation results realign to in-slot order via a bf16 DRAM round trip
  (PE transpose -> rows -> indirect gather -> PE transpose).
- Fusion: out.T = relu(W_fuse @ [in_f; out_f].T) in transposed layout (bf16
  matmuls, f32 psum); host reassembles rows.

History (TimelineSim ns, which matched the measured fp32 baseline to 0.4%):
  fp32 baseline 1368320 -> f32r wide tiles 803908 -> bf16 + host-marshaled
  posT + COLTILE=256 double-buffered PSUM 549397 -> interleaved ef-build /
  deferred weights 461234 -> two-phase act emission (no head-of-line
  blocking) + bf16 act outputs 378184 -> 8-chunk posT DMA 374726 ->
  deeper work/stage pools + out-ef pacing 362535 -> c*=sig(f)
  on gpsimd + interleaved posT chunk order 358260 -> chunked fusion
  tail overlapping gather dispatch 356645 -> first-step
  specialization (h=0: skip Whh, c=0: skip sig(f)/cmul/cadd) 354321 ->
  tanh(g) via sigma table (host doubles g-gate weights; tanh(g)=2*sig(2g)-1
  reconstructed with one fused scalar_tensor_tensor on DVE) 327792 ->
  out-agg runs first so the d_rows realign round trip hides inside the
  in recurrence via an emission hook; tail is just fusion 323990.
  HW-verified rel err 1.01e-2 (gate 2e-2). PSUM gotcha: matmul start=True
  zeroes the whole 2KB bank, so with two gate regions per bank only the
  bank's first matmul may set start (see emit_wih).
"""
import sys
sys.path.insert(0, '/opt/trn_rl_repo')

import numpy as np
import ml_dtypes
from contextlib import ExitStack

import concourse.bass as bass
import concourse.tile as tile
import concourse.mybir as mb
from concourse import mybir
from concourse.bass_utils import run_bass_kernel_spmd
from concourse.masks import make_identity

N_NODES = 8000
N_EDGES = 80000
EDGE_DIM = 8
H = 256
HH = 128
MAX_LEN = 5000
NC = 8
F32 = mybir.dt.float32
BF16 = mybir.dt.bfloat16
I32 = mybir.dt.int32
BF16NP = ml_dtypes.bfloat16

C = 256  # recurrence column tile; 4 gates x 256 f32 = one 2-bank PSUM tile


# ---------------------------------------------------------------------------
# walrus in this container encodes at most ONE sync-wait per instruction.
def fix_sync_waits(nc):
    templates = {}
    tmpl_names = set()
    for engname in ("sync", "tensor", "scalar", "vector", "gpsimd"):
        t = getattr(nc, engname).nop()
        templates[t.ins.engine] = t.ins
        tmpl_names.add(t.ins.name)
    ctr = 0
    for f in nc.m.functions:
        for bb in f.blocks:
            il = bb.instructions
            out = []
            changed = False
            for ins in il:
                if ins.name in tmpl_names:
                    changed = True
                    continue
                si = ins.sync_info
                if si is not None and len(si.on_wait) > 1:
                    waits = list(si.on_wait)
                    tmpl = templates[ins.engine]
                    for w in waits[:-1]:
                        out.append(tmpl.__replace__(
                            name=f"waitnop-{ctr}",
                            sync_info=mb.SyncInfo(on_wait=[w], on_update=[]),
                        ))
                        ctr += 1
                    ins.sync_info = mb.SyncInfo(
                        on_wait=[waits[-1]], on_update=list(si.on_update))
                    changed = True
                out.append(ins)
            if changed:
                bb.instructions = out


# ---------------------------------------------------------------------------
def _buckets(key, num_nodes):
    """edge ids per node, original order preserved (stable)."""
    counts = np.bincount(key, minlength=num_nodes)
    order = np.argsort(key, kind='stable')
    starts = np.zeros(num_nodes + 1, np.int64)
    starts[1:] = np.cumsum(counts)
    return order, starts, counts


def _prep_agg(key, edge_attr, node_core, cores=NC):
    """Host marshaling for one aggregator. Returns per-core arrays + schedule."""
    order, starts, deg = _buckets(key, N_NODES)
    dmax = int(deg.max())

    # per-core nodes and per-degree counts
    core_nodes = [np.where(node_core == c)[0] for c in range(cores)]
    cnt = np.zeros((cores, dmax + 1), np.int64)
    for c in range(cores):
        cnt[c] = np.bincount(deg[core_nodes[c]], minlength=dmax + 1)
    common = cnt.max(axis=0)          # common[v] slots of degree v (v>=1 used)

    # slot -> node (or -1) per core, degree descending; then degree-0 region
    prof = []                          # slot degree profile (v>=1)
    for v in range(dmax, 0, -1):
        prof.extend([v] * int(common[v]))
    prof = np.array(prof, np.int32)
    n_prof = len(prof)
    deg0_max = int(cnt[:, 0].max())
    S = n_prof + deg0_max
    S128 = -(-S // 128) * 128

    slot_node = np.full((cores, S128), -1, np.int64)
    for c in range(cores):
        pos = 0
        for v in range(dmax, 0, -1):
            nn = core_nodes[c][deg[core_nodes[c]] == v]
            slot_node[c, pos:pos + len(nn)] = np.sort(nn)
            pos += int(common[v])
        z = core_nodes[c][deg[core_nodes[c]] == 0]
        slot_node[c, n_prof:n_prof + len(z)] = np.sort(z)

    # schedule
    B = [int((prof > t).sum()) for t in range(dmax)]
    Ec = int(sum(B))
    Ec128 = -(-Ec // 128) * 128
    Ec512 = -(-Ec128 // 512) * 512
    off = np.zeros(dmax + 1, np.int64)
    off[1:] = np.cumsum(B)

    # step-major edge list per core (edge id or -1)
    esm = np.full((cores, Ec512), -1, np.int64)
    for c in range(cores):
        col = 0
        for t in range(dmax):
            sl = slot_node[c, :B[t]]
            real = sl >= 0
            e = np.full(B[t], -1, np.int64)
            e[real] = order[starts[sl[real]] + t]
            esm[c, col:col + B[t]] = e
            col += B[t]

    # marshaled attrs (bf16, transposed)
    attrT = np.zeros((cores, EDGE_DIM, Ec512), BF16NP)
    for c in range(cores):
        e = esm[c]
        real = e >= 0
        a = np.zeros((Ec512, EDGE_DIM), np.float32)
        a[real] = edge_attr[e[real]]
        attrT[c] = a.T.astype(BF16NP)

    # degree-1 slot range (for LSTM bypass)
    d1a = int((prof > 1).sum())
    d1b = d1a + int(common[1] if dmax >= 1 else 0)

    # node -> slot map per core
    node_slot = np.full((cores, N_NODES), 0, np.int64)
    for c in range(cores):
        real = slot_node[c] >= 0
        node_slot[c, slot_node[c][real]] = np.where(real)[0]

    return dict(dmax=dmax, B=B, off=off, Ec=Ec, Ec512=Ec512, S=S, S128=S128,
                slot_node=slot_node, node_slot=node_slot, esm=esm,
                attrT=attrT, d1=(d1a, d1b))


def _host_prep(edge_index, edge_attr, edge_timestamps):
    src = np.asarray(edge_index[0]); dst = np.asarray(edge_index[1])
    din = np.bincount(dst, minlength=N_NODES)
    dout = np.bincount(src, minlength=N_NODES)

    # positions (exact fp32 replica of the reference arithmetic)
    ts = np.asarray(edge_timestamps, np.float32)
    tmin = ts.min(); tmax = ts.max()
    if tmax > tmin:
        denom = np.float32(tmax - tmin)
        positions = ((ts - tmin) / denom * np.float32(4999.0)).astype(np.int32)
    else:
        positions = np.zeros(N_EDGES, np.int32)

    # deal nodes to cores balanced on (din, dout)
    lex = np.lexsort((np.arange(N_NODES), dout, din))
    node_core = np.empty(N_NODES, np.int64)
    node_core[lex] = np.arange(N_NODES) % NC

    A_in = _prep_agg(dst, edge_attr, node_core)
    A_out = _prep_agg(src, edge_attr, node_core)

    S = max(A_in['S'], A_out['S'])
    S128 = -(-S // 128) * 128
    for A in (A_in, A_out):
        if A['S128'] != S128:
            pad = np.full((NC, S128 - A['S128']), -1, np.int64)
            A['slot_node'] = np.concatenate([A['slot_node'], pad], axis=1)
        A['S128'] = S128

    # fusion realignment: for in-slot j -> out-slot of the same node
    fus = np.zeros((NC, 128, S128 // 128), np.int32)
    for c in range(NC):
        sl = A_in['slot_node'][c]
        f = np.zeros(S128, np.int64)
        real = sl >= 0
        f[real] = A_out['node_slot'][c, sl[real]]
        fus[c] = f.reshape(-1, 128).T
    return A_in, A_out, fus, node_core, S128, positions


def _make_posT(A, positions, pos_emb_ts, core):
    """[128, 2*Ec512] bf16: posT[p, k*Ec512+e] = ts*pos_emb[pos[e], k*128+p]."""
    Ec512 = A['Ec512']
    e = A['esm'][core]
    real = e >= 0
    p = np.zeros(Ec512, np.int64)
    p[real] = positions[e[real]]
    g = pos_emb_ts[p]                      # [Ec512, 256] f32
    g[~real] = 0.0
    return np.ascontiguousarray(
        g.T.reshape(2, 128, Ec512).transpose(1, 0, 2).reshape(128, 2 * Ec512)
    ).astype(BF16NP)


def build_in_maps(inp, A_in, A_out, fus, positions):
    """Per-core input maps from full inputs (host marshaling)."""
    tsv = np.float32(np.asarray(inp['time_scale'], np.float32).reshape(-1)[0])
    pos_emb_ts = np.asarray(inp['pos_emb'], np.float32) * tsv
    def prep_w(w):
        # transpose to [dir, in, 4h]; double the g-gate block so the device
        # computes sigma(2g), and tanh(g) = 2*sigma(2g) - 1
        wt = np.ascontiguousarray(np.transpose(np.asarray(w, np.float32), (0, 2, 1))).copy()
        wt[:, :, 2 * HH:3 * HH] *= 2.0
        return wt
    wih = {"in": prep_w(inp['in_Wih']), "out": prep_w(inp['out_Wih'])}
    whh = {"in": prep_w(inp['in_Whh']), "out": prep_w(inp['out_Whh'])}
    maps = []
    for c in range(NC):
        maps.append({
            "w_projT": np.ascontiguousarray(np.asarray(inp['W_proj'], np.float32).T),
            "w_fuseT": np.ascontiguousarray(np.asarray(inp['W_fuse'], np.float32).T),
            "attrT_in": A_in['attrT'][c], "attrT_out": A_out['attrT'][c],
            "posT_in": _make_posT(A_in, positions, pos_emb_ts, c),
            "posT_out": _make_posT(A_out, positions, pos_emb_ts, c),
            "wihT_in": wih["in"], "wihT_out": wih["out"],
            "whhT_in": whh["in"], "whhT_out": whh["out"],
            "fusidx": fus[c],
        })
    return maps


# ---------------------------------------------------------------------------
def _build_device(A_in, A_out, S128, biases_zero, waitfix=True, reps=1):
    assert biases_zero, "nonzero LSTM/proj biases not implemented"
    nc = bass.Bass()

    def param(name, shape, dt=F32):
        return nc.declare_dram_parameter(name, list(shape), dt, isOutput=False)

    p_wproj = param("w_projT", [EDGE_DIM, H])
    p_wfuse = param("w_fuseT", [2 * H, 2 * H])
    p_attr = {a: param(f"attrT_{a}", [EDGE_DIM, A['Ec512']], BF16)
              for a, A in (("in", A_in), ("out", A_out))}
    p_pos = {a: param(f"posT_{a}", [128, 2 * A['Ec512']], BF16)
             for a, A in (("in", A_in), ("out", A_out))}
    p_wih = {a: param(f"wihT_{a}", [2, H, 4 * HH]) for a in ("in", "out")}
    p_whh = {a: param(f"whhT_{a}", [2, HH, 4 * HH]) for a in ("in", "out")}
    p_fus = param("fusidx", [128, S128 // 128], I32)
    p_y = nc.declare_dram_parameter("y", [4, 128, S128], F32, isOutput=True)
    d_rows = nc.dram_tensor("out_rows", [S128, 2 * HH], BF16)

    # psum gate region r (order i,f,o,g) <- weight col range (order i,f,g,o)
    wslice = [slice(0, 128), slice(128, 256), slice(384, 512), slice(256, 384)]

    with tile.TileContext(nc) as tc, ExitStack() as ctx:
        const = ctx.enter_context(tc.tile_pool(name="const", bufs=1))
        wpool = ctx.enter_context(tc.tile_pool(name="w", bufs=1))
        efp = ctx.enter_context(tc.tile_pool(name="ef", bufs=1))
        stp = ctx.enter_context(tc.tile_pool(name="stage", bufs=4))
        state = ctx.enter_context(tc.tile_pool(name="state", bufs=1))
        work = ctx.enter_context(tc.tile_pool(name="work", bufs=5))
        # PSUM: four 2-bank tiles, rotating
        psg = ctx.enter_context(tc.tile_pool(name="psg", bufs=1, space="PSUM"))
        ptags = ["p0", "p1", "p2", "p3"]
        pctr = [0]

        def ptile(name):
            t = psg.tile([128, 1024], F32, tag=ptags[pctr[0] % 4], name=name)
            pctr[0] += 1
            return t

        ident = const.tile([128, 128], F32)
        make_identity(nc, ident[:])
        identb = const.tile([128, 128], BF16)
        nc.vector.tensor_copy(identb[:], ident[:])
        wproj32 = stp.tile([EDGE_DIM, H], F32, tag="wstage", name="wproj32")
        nc.sync.dma_start(wproj32[:], p_wproj.ap())
        wproj = const.tile([EDGE_DIM, H], BF16)
        nc.vector.tensor_copy(wproj[:], wproj32[:])
        fusidx = const.tile([128, S128 // 128], I32)
        nc.sync.dma_start(fusidx[:], p_fus.ap())

        wih = {}; whh = {}; wfuse = []

        def load_agg_weights(a, eng=None):
            cp = (eng or nc.scalar).copy if eng is not nc.vector else nc.vector.tensor_copy
            for d in range(2):
                for k in range(2):
                    t = wpool.tile([128, 512], BF16, tag=f"wih{a}{d}{k}", name=f"wih{a}{d}{k}")
                    wstage = stp.tile([128, 512], F32, tag="wstage", name=f"wis{a}{d}{k}")
                    nc.sync.dma_start(wstage[:], p_wih[a].ap()[d, k * 128:(k + 1) * 128, :])
                    cp(t[:], wstage[:])
                    wih[(a, d, k)] = t
                t = wpool.tile([128, 512], BF16, tag=f"whh{a}{d}", name=f"whh{a}{d}")
                wstage = stp.tile([128, 512], F32, tag="wstage", name=f"whs{a}{d}")
                nc.sync.dma_start(wstage[:], p_whh[a].ap()[d])
                cp(t[:], wstage[:])
                whh[(a, d)] = t

        def load_fuse_weights():
            wfuse.clear()
            for k in range(4):
                t = wpool.tile([128, 512], BF16, tag=f"wf{k}", name=f"wf{k}")
                wstage = stp.tile([128, 512], F32, tag="wstage", name=f"wfs{k}")
                nc.sync.dma_start(wstage[:], p_wfuse.ap()[k * 128:(k + 1) * 128, :])
                nc.vector.tensor_copy(t[:], wstage[:])
                wfuse.append(t)

        # timing variants repeat the whole body; tile names auto-uniquify
        for _rep in range(reps):
            results = {}

            load_agg_weights("out")
            # ---- per-agg ef state: tiles, DMAs (queued up-front), block emitter
            NCHK = 8  # posT DMA chunks per k-plane
            actx = {}
            for a in ("out", "in"):
                A = A_in if a == "in" else A_out
                Ec512 = A['Ec512']
                efT = efp.tile([128, 2 * Ec512], BF16, tag=f"posT_{a}",
                               name=f"efT_{a}_{_rep}")
                att = efp.tile([EDGE_DIM, Ec512], BF16, tag=f"attr_{a}",
                               name=f"attr_{a}_{_rep}")
                nc.sync.dma_start(att[:], p_attr[a].ap())
                ck = Ec512 // NCHK
                # chunk order: alternate front (fwd chain) / back (bwd chain)
                qorder = []
                lo, hi = 0, NCHK - 1
                while lo <= hi:
                    qorder.append(lo); lo += 1
                    if lo <= hi:
                        qorder.append(hi); hi -= 1
                for q in qorder:
                    for k in range(2):
                        sl = slice(k * Ec512 + q * ck, k * Ec512 + (q + 1) * ck)
                        nc.sync.dma_start(efT[:, sl], p_pos[a].ap()[:, sl])
                nblk = -(-A['Ec'] // 512)   # pad cols are never read
                actx[a] = dict(A=A, efT=efT, att=att, done=[False] * nblk,
                               nblk=nblk)

            def emit_ef_block(a, b):
                cx = actx[a]
                if b >= cx['nblk'] or cx['done'][b]:
                    return
                cx['done'][b] = True
                Ec512 = cx['A']['Ec512']
                c0 = b * 512
                ps = ptile(f"efps_{a}_{c0}_{_rep}")
                for k in range(2):
                    nc.tensor.matmul(ps[:, k * 512:(k + 1) * 512],
                                     lhsT=wproj[:, k * 128:(k + 1) * 128],
                                     rhs=cx['att'][:, c0:c0 + 512],
                                     start=True, stop=True)
                for k in range(2):
                    ef_sl = cx['efT'][:, k * Ec512 + c0: k * Ec512 + c0 + 512]
                    nc.vector.tensor_add(ef_sl, ef_sl, ps[:, k * 512:(k + 1) * 512])

            def ensure_ef(a, t):
                cx = actx[a]
                off = cx['A']['off']; B = cx['A']['B']
                for b in range(int(off[t]) // 512,
                               -(-int(off[t] + B[t]) // 512)):
                    emit_ef_block(a, b)

            # out-ef blocks paced into the second half of in-rec (front/back
            # alternating so both out chains can start immediately after)
            def out_ef_order(a):
                n = actx[a]['nblk']
                lo, hi = 0, n - 1
                order = []
                while lo <= hi:
                    order.append(lo); lo += 1
                    if lo <= hi:
                        order.append(hi); hi -= 1
                return order

            hsave = {}

            def run_agg(a, hook):
                A = actx[a]['A']; efT = actx[a]['efT']
                Ec512 = A['Ec512']; dmax = A['dmax']; B = A['B']; off = A['off']
                hs = {}; cs = {}
                for d, nm in ((0, "f"), (1, "b")):
                    hs[d] = state.tile([128, S128], BF16, tag=f"h_{a}_{nm}",
                                       name=f"h_{a}_{nm}_{_rep}")
                    nc.gpsimd.memset(hs[d][:], 0.0)
                    cs[d] = state.tile([128, S128], F32, tag=f"c_{nm}",
                                       name=f"c_{a}_{nm}_{_rep}")
                    nc.gpsimd.memset(cs[d][:], 0.0)

                def emit_wih(d, t, c0, w, gt, first=False):
                    # start=True resets the WHOLE psum bank: only the first
                    # matmul touching each bank (regions r=0,2) may set it;
                    # the bank-mate region (r=1,3) accumulates onto the zeroed
                    # bank with start=False. On the chain's first step h=0, so
                    # the Whh pass is skipped and k=1 closes the group.
                    col = int(off[t]) + c0
                    for k in range(2):
                        for r in range(4):
                            nc.tensor.matmul(
                                gt[:, r * C: r * C + w],
                                lhsT=wih[(a, d, k)][:, wslice[r]],
                                rhs=efT[:, k * Ec512 + col: k * Ec512 + col + w],
                                start=(k == 0 and r % 2 == 0),
                                stop=(first and k == 1),
                                skip_group_check=True)

                def emit_whh(d, t, c0, w, gt):
                    h = hs[d]
                    for r in range(4):
                        nc.tensor.matmul(
                            gt[:, r * C: r * C + w],
                            lhsT=whh[(a, d)][:, wslice[r]],
                            rhs=h[:, c0:c0 + w],
                            start=False, stop=True, skip_group_check=True)

                def emit_act1(d, t, c0, w, gt, first=False):
                    c = cs[d]
                    csl = c[:, c0:c0 + w]
                    if first:
                        # c_prev = 0: c = sig(i)*(2*sig(2g)-1); sig(f) unneeded
                        sifo4 = work.tile([128, 4 * C], BF16, tag="sifo4")
                        nc.scalar.activation(
                            out=sifo4[:].rearrange("p (r x) -> p r x", r=4)[:, 0:4:2, 0:w],
                            in_=gt[:].rearrange("p (r x) -> p r x", r=4)[:, 0:4:2, 0:w],
                            func=mybir.ActivationFunctionType.Sigmoid)
                        nc.scalar.activation(
                            out=sifo4[:, 3 * C:3 * C + w],
                            in_=gt[:, 3 * C:3 * C + w],
                            func=mybir.ActivationFunctionType.Sigmoid)
                        si = sifo4[:, 0:w]
                        sg = sifo4[:, 3 * C:3 * C + w]
                        tmp = work.tile([128, C], BF16, tag="tmp")
                        nc.vector.tensor_mul(tmp[:, 0:w], si, sg)
                        nc.vector.scalar_tensor_tensor(
                            csl, tmp[:, 0:w], 2.0, si,
                            op0=mybir.AluOpType.mult, op1=mybir.AluOpType.subtract)
                        return sifo4
                    sifo4 = work.tile([128, 4 * C], BF16, tag="sifo4")
                    nc.scalar.activation(
                        out=sifo4[:].rearrange("p (r x) -> p r x", r=4)[:, :, 0:w],
                        in_=gt[:].rearrange("p (r x) -> p r x", r=4)[:, :, 0:w],
                        func=mybir.ActivationFunctionType.Sigmoid)
                    si = sifo4[:, 0:w]
                    sf = sifo4[:, C:C + w]
                    sg = sifo4[:, 3 * C:3 * C + w]
                    tmp = work.tile([128, C], BF16, tag="tmp")
                    nc.vector.tensor_mul(tmp[:, 0:w], si, sg)
                    v = work.tile([128, C], BF16, tag="tg")
                    nc.vector.scalar_tensor_tensor(
                        v[:, 0:w], tmp[:, 0:w], 2.0, si,
                        op0=mybir.AluOpType.mult, op1=mybir.AluOpType.subtract)
                    nc.gpsimd.tensor_mul(csl, csl, sf)
                    nc.vector.tensor_add(csl, csl, v[:, 0:w])
                    return sifo4

                def emit_act2(d, t, c0, w, sifo):
                    h, c = hs[d], cs[d]
                    so = sifo[:, 2 * C:2 * C + w]
                    csl = c[:, c0:c0 + w]
                    tc_ = work.tile([128, C], BF16, tag="tc")
                    nc.scalar.activation(out=tc_[:, 0:w], in_=csl,
                                         func=mybir.ActivationFunctionType.Tanh)
                    nc.vector.tensor_mul(h[:, c0:c0 + w], so, tc_[:, 0:w])

                for i in range(dmax):
                    tf, tb = i, dmax - 1 - i
                    ensure_ef(a, tf)
                    ensure_ef(a, tb)
                    if hook:
                        hook(i)
                    ctf = [(c0, min(C, B[tf] - c0)) for c0 in range(0, B[tf], C)]
                    ctb = [(c0, min(C, B[tb] - c0)) for c0 in range(0, B[tb], C)]
                    n = max(len(ctf), len(ctb))
                    # coltile-pair interleave: two Wih pairs, then their Whh+act
                    fr = (i == 0)   # chains' first step: h=0, c=0
                    pend = []
                    for j in range(n):
                        batch = []
                        if j < len(ctf):
                            gt = ptile(f"g_{a}_f_{i}_{j}_{_rep}")
                            emit_wih(0, tf, ctf[j][0], ctf[j][1], gt, fr)
                            batch.append((0, tf, ctf[j][0], ctf[j][1], gt, fr))
                        if j < len(ctb):
                            gt = ptile(f"g_{a}_b_{i}_{j}_{_rep}")
                            emit_wih(1, tb, ctb[j][0], ctb[j][1], gt, fr)
                            batch.append((1, tb, ctb[j][0], ctb[j][1], gt, fr))
                        for (d, t, c0, w, gt, f0) in pend:
                            if not f0:
                                emit_whh(d, t, c0, w, gt)
                        sif = [emit_act1(d, t, c0, w, gt, f0)
                               for (d, t, c0, w, gt, f0) in pend]
                        for (d, t, c0, w, gt, f0), s in zip(pend, sif):
                            emit_act2(d, t, c0, w, s)
                        pend = batch
                    for (d, t, c0, w, gt, f0) in pend:
                        if not f0:
                            emit_whh(d, t, c0, w, gt)
                    sif = [emit_act1(d, t, c0, w, gt, f0)
                           for (d, t, c0, w, gt, f0) in pend]
                    for (d, t, c0, w, gt, f0), s in zip(pend, sif):
                        emit_act2(d, t, c0, w, s)

                # degree-1 bypass: slots [d1a, d1b) -> ef of their single edge
                d1a, d1b = A['d1']
                if d1b > d1a:
                    nc.vector.tensor_copy(hs[0][:, d1a:d1b], efT[:, d1a:d1b])
                    nc.vector.tensor_copy(hs[1][:, d1a:d1b],
                                          efT[:, Ec512 + d1a:Ec512 + d1b])
                hsave[a] = (hs[0], hs[1])

            oorder = out_ef_order("in")

            def out_hook(i):
                # pace in-ef blocks into out-rec once in posT DMAs landed
                if i >= 12:
                    take = 2
                    while take and oorder:
                        emit_ef_block("in", oorder.pop(0))
                        take -= 1

            run_agg("out", out_hook)
            load_agg_weights("in", eng=nc.vector)
            load_fuse_weights()
            for b in oorder:
                emit_ef_block("in", b)

            # hrow + realign state, emitted via hook inside in-rec so the
            # d_rows round trip hides under the in recurrence
            hs_o = hsave["out"]
            ot0 = state.tile([128, S128], BF16, tag="ot0", name=f"ot0_{_rep}")
            ot1 = state.tile([128, S128], BF16, tag="ot1", name=f"ot1_{_rep}")
            nj = S128 // 128
            fg = efp.tile([128, nj * 256], BF16, tag="fgall", name=f"fgall_{_rep}")

            def emit_hrow(j):
                tp = ptile(f"hrow_{j}_{_rep}")[:].bitcast(BF16)
                nc.tensor.transpose(out=tp[:, 0:128],
                                    in_=hs_o[0][:, j * 128:(j + 1) * 128],
                                    identity=identb[:])
                nc.tensor.transpose(out=tp[:, 128:256],
                                    in_=hs_o[1][:, j * 128:(j + 1) * 128],
                                    identity=identb[:])
                row = stp.tile([128, 256], BF16, tag="row")
                nc.vector.tensor_copy(row[:], tp[:, 0:256])
                nc.sync.dma_start(d_rows[j * 128:(j + 1) * 128, :], row[:])

            def emit_realign(j):
                tp = ptile(f"fgrow_{j}_{_rep}")[:].bitcast(BF16)
                nc.tensor.transpose(out=tp[:, 0:128],
                                    in_=fg[:, j * 256:j * 256 + 128],
                                    identity=identb[:])
                nc.tensor.transpose(out=tp[:, 128:256],
                                    in_=fg[:, j * 256 + 128:(j + 1) * 256],
                                    identity=identb[:])
                nc.vector.tensor_copy(ot0[:, j * 128:(j + 1) * 128], tp[:, 0:128])
                nc.vector.tensor_copy(ot1[:, j * 128:(j + 1) * 128], tp[:, 128:256])

            state_ih = {'h': 0, 'g': 0, 'r': 0}

            def in_hook(i):
                if i >= 1 and state_ih['h'] < nj:
                    emit_hrow(state_ih['h'])
                    state_ih['h'] += 1
                elif state_ih['h'] >= nj and state_ih['g'] == 0:
                    for j in range(nj):
                        nc.gpsimd.indirect_dma_start(
                            out=fg[:, j * 256:(j + 1) * 256], out_offset=None,
                            in_=d_rows[:],
                            in_offset=bass.IndirectOffsetOnAxis(
                                ap=fusidx[:, j:j + 1], axis=0))
                    state_ih['g'] = 1
                elif state_ih['g'] and state_ih['r'] < nj:
                    emit_realign(state_ih['r'])
                    state_ih['r'] += 1

            run_agg("in", in_hook)
            for j in range(state_ih['h'], nj):
                emit_hrow(j)
            if not state_ih['g']:
                for j in range(nj):
                    nc.gpsimd.indirect_dma_start(
                        out=fg[:, j * 256:(j + 1) * 256], out_offset=None,
                        in_=d_rows[:],
                        in_offset=bass.IndirectOffsetOnAxis(
                            ap=fusidx[:, j:j + 1], axis=0))
            for j in range(state_ih['r'], nj):
                emit_realign(j)

            # ---- fusion: all parts ready
            in_f, in_b = hsave["in"]
            parts = [in_f, in_b, ot0, ot1]
            for c0 in range(0, S128, 512):
                w = min(512, S128 - c0)
                for m in range(4):
                    ps = ptile(f"fus_{m}_{c0}_{_rep}")
                    for k in range(4):
                        nc.tensor.matmul(ps[:, 0:w],
                                         lhsT=wfuse[k][:, m * 128:(m + 1) * 128],
                                         rhs=parts[k][:, c0:c0 + w],
                                         start=(k == 0), stop=(k == 3))
                    o = work.tile([128, 512], F32, tag="fo")
                    nc.scalar.activation(out=o[:, 0:w], in_=ps[:, 0:w],
                                         func=mybir.ActivationFunctionType.Relu)
                    nc.sync.dma_start(p_y.ap()[m][:, c0:c0 + w], o[:, 0:w])

    if waitfix:
        fix_sync_waits(nc)
    return nc


# ---------------------------------------------------------------------------
_CACHE = {}


def _get_built(edge_index, edge_attr, edge_timestamps, biases_zero, waitfix=True):
    key = hash((edge_index.tobytes(), biases_zero, waitfix))
    if key not in _CACHE:
        A_in, A_out, fus, node_core, S128, positions = _host_prep(
            edge_index, edge_attr, edge_timestamps)
        nc = _build_device(A_in, A_out, S128, biases_zero, waitfix=waitfix)
        _CACHE[key] = (A_in, A_out, fus, node_core, S128, positions, nc)
    return _CACHE[key]


def kernel(edge_index, edge_attr, edge_timestamps, W_proj, b_proj, pos_emb,
           time_scale, in_Wih, in_Whh, in_bih, in_bhh,
           out_Wih, out_Whh, out_bih, out_bhh, W_fuse, b_fuse):
    edge_index = np.asarray(edge_index)
    edge_attr = np.asarray(edge_attr, np.float32)
    edge_timestamps = np.asarray(edge_timestamps, np.float32)
    biases_zero = all(not np.any(np.asarray(x)) for x in
                      (b_proj, in_bih, in_bhh, out_bih, out_bhh, b_fuse))
    A_in, A_out, fus, node_core, S128, positions, nc = _get_built(
        edge_index, edge_attr, edge_timestamps, biases_zero)

    inp = dict(pos_emb=pos_emb, time_scale=time_scale, W_proj=W_proj,
               W_fuse=W_fuse, in_Wih=in_Wih, in_Whh=in_Whh,
               out_Wih=out_Wih, out_Whh=out_Whh)
    in_maps = build_in_maps(inp, A_in, A_out, fus, positions)
    res = run_bass_kernel_spmd(nc, in_maps, list(range(NC)), trace=False)

    out = np.zeros((N_NODES, 2 * H), np.float32)
    for c in range(NC):
        y = res.results[c]["y"]              # [4, 128, S128]
        sl = A_in['slot_node'][c]
        real = sl >= 0
        js = np.where(real)[0]
        out[sl[js]] = y[:, :, js].reshape(512, len(js)).T
    return out
